# revision 1
# baseline (speedup 1.0000x reference)
"""Causal multi-head self-attention (B=32, T=512, C=1024, H=16) on 8 trn2 cores.

Strategy: data-parallel over batch (4 items/core), identical NEFF on all
cores.  All activations are kept in [channel, token] layout on device so
every matmul has its contraction dim on partitions with no transposes:

  QT/KT  = W^T-tiles.T @ xT-tiles           (fp32r, full speed at N=512)
  S_T    = K_slice.T @ Q_slice  [k, q]      (fp32r; causal => shrink N per kt;
                                             head pairs run concurrently in
                                             PE row groups 0-1 / 2-3)
  att    = exp(scale*S_T + pad_bias[k])     (ACT; pad mask as per-partition bias)
  att   *= causal_binmask (diag block only) (GpSimd, 0/1 bf16 multiply)
  y/den  = [V | 1].T @ att                  (bf16; ones column gives softmax denom)
  1/den  = exp(-ln(den))                    (ACT, [1,512], straight from PSUM)
  yT     = y * bcast(1/den)                 (GpSimd partition_broadcast + DVE mult)
  outT   = Wp^T-tiles.T @ yT + bp_eff       (bf16)

bq/bk are fused into the PSUM evacuation; bv is folded into
bp_eff = bp + Wp @ bv on the host (valid because softmax rows sum to 1).
Batches are software-pipelined: proj(b) is emitted after the QKV
projections of batch b+1 so the PE never idles while the softmax
denominator chain (ACT/GpSimd/DVE) finishes.
"""

import sys

sys.path.insert(0, "/opt/trn_rl_repo")

import ml_dtypes
import numpy as np

import concourse.bass as bass
import concourse.tile as tile
from concourse import bacc, mybir

B, T, C, H = 32, 512, 1024, 16
D = C // H  # 64
N_CORES = 8
BL = B // N_CORES  # batches per core
NEG = -1.0e9

F32 = mybir.dt.float32
F32R = mybir.dt.float32r
BF16 = mybir.dt.bfloat16
BF16_NP = ml_dtypes.bfloat16
AF = mybir.ActivationFunctionType
OP = mybir.AluOpType


def build_nc(c=C, t=T, bl=BL, h=H):
    """Build the per-core Bass program. Same NEFF runs on every core."""
    nct = c // 128   # channel tiles
    ktt = t // 128   # key/token tiles per sequence
    nch = (c + 511) // 512  # 512-wide output chunks for V projection
    scale = 1.0 / float(np.sqrt(D))

    nc = bacc.Bacc(None, target_bir_lowering=False)

    xT = nc.dram_tensor("xT", [c, bl * t], F32R, kind="ExternalInput")
    xTb = nc.dram_tensor("xTb", [c, bl * t], BF16, kind="ExternalInput")
    wq_t = nc.dram_tensor("wq_t", [c, c], F32R, kind="ExternalInput")
    wk_t = nc.dram_tensor("wk_t", [c, c], F32R, kind="ExternalInput")
    wv_t = nc.dram_tensor("wv_t", [c, c], BF16, kind="ExternalInput")
    wp_t = nc.dram_tensor("wp_t", [c, c], BF16, kind="ExternalInput")
    bq_t = nc.dram_tensor("bq_t", [128, nct], F32, kind="ExternalInput")
    bk_t = nc.dram_tensor("bk_t", [128, nct], F32, kind="ExternalInput")
    bpe_t = nc.dram_tensor("bpe_t", [128, nct], F32, kind="ExternalInput")
    pad_t = nc.dram_tensor("pad_t", [128, bl * ktt], F32, kind="ExternalInput")
    cmask = nc.dram_tensor("cmask", [128, 2, 128], BF16, kind="ExternalInput")
    outT = nc.dram_tensor("outT", [bl, c, t], F32, kind="ExternalOutput")
    # DRAM scratch for the reciprocal fold/broadcast (ExternalOutput because
    # Internal DRAM tensors fail to load under the PJRT runtime path)
    scr = nc.dram_tensor("scr", [bl, h, 2, t], F32, kind="ExternalOutput")

    with tile.TileContext(nc) as tc:
        with (
            tc.tile_pool(name="weights", bufs=1) as wpool,
            tc.tile_pool(name="consts", bufs=1) as cpool,
            tc.tile_pool(name="acts", bufs=1) as apool,
            tc.tile_pool(name="att", bufs=8) as attp,
            tc.tile_pool(name="norm", bufs=3) as npool,
            tc.tile_pool(name="oevac", bufs=2) as opool,
            tc.tile_pool(name="psum", bufs=4, space=bass.MemorySpace.PSUM) as pp,
        ):
            # ---- load weights / constants once (per-k-tile DMAs) ----
            wq_sb = wpool.tile([128, nct, c], F32R, tag="wq")
            wk_sb = wpool.tile([128, nct, c], F32R, tag="wk")
            wv_sb = wpool.tile([128, nct, c], BF16, tag="wv")
            wp_sb = wpool.tile([128, nct, c], BF16, tag="wp")
            for w_sb, w_dr in ((wq_sb, wq_t), (wk_sb, wk_t), (wv_sb, wv_t), (wp_sb, wp_t)):
                w_r = w_dr[:].rearrange("(k p) m -> p k m", p=128)
                for k in range(nct):
                    nc.sync.dma_start(w_sb[:, k, :], w_r[:, k, :])

            bq_sb = cpool.tile([128, nct], F32, tag="bq")
            bk_sb = cpool.tile([128, nct], F32, tag="bk")
            bpe_sb = cpool.tile([128, nct], F32, tag="bpe")
            pad_sb = cpool.tile([128, bl * ktt], F32, tag="pad")
            cm_sb = cpool.tile([128, 2, 128], BF16, tag="cmask")
            nc.sync.dma_start(bq_sb, bq_t[:])
            nc.sync.dma_start(bk_sb, bk_t[:])
            nc.sync.dma_start(bpe_sb, bpe_t[:])
            nc.sync.dma_start(pad_sb, pad_t[:])
            nc.sync.dma_start(cm_sb, cmask[:])

            yT_tiles = [None, None]  # per-parity yT tiles (bufs=2 pipelining)

            def emit_proj_group(yT_sb, b, m):
                ps = pp.tile([128, t], F32, tag="ps", name=f"pj{b}_{m}")
                for k in range(nct):
                    nc.tensor.matmul(
                        ps,
                        wp_sb[:, k, m * 128 : (m + 1) * 128],
                        yT_sb[:, k, :],
                        start=(k == 0),
                        stop=(k == nct - 1),
                    )
                ot = opool.tile([128, t], F32, tag="ot", name=f"ot{b}_{m}")
                nc.scalar.activation(
                    ot, ps, AF.Identity, bias=bpe_sb[:, m : m + 1]
                )
                nc.sync.dma_start(outT[b, m * 128 : (m + 1) * 128, :], ot)

            pending_proj = None  # (yT_sb, b, next_m) — interleaved into attn

            for b in range(bl):
                # ---- load this batch's activations (per-k-tile DMAs) ----
                x_sb = apool.tile([128, nct, t], F32R, tag="x")
                xb_sb = apool.tile([128, nct, t], BF16, tag="xb")
                x_r = xT[:, b * t : (b + 1) * t].rearrange("(k p) n -> p k n", p=128)
                xb_r = xTb[:, b * t : (b + 1) * t].rearrange("(k p) n -> p k n", p=128)
                for k in range(nct):
                    nc.sync.dma_start(x_sb[:, k, :], x_r[:, k, :])
                    nc.sync.dma_start(xb_sb[:, k, :], xb_r[:, k, :])

                # ---- Q/K projections (fp32r) -> [c, t] layout; evac on ACT ----
                qT_sb = apool.tile([128, nct, t], F32R, tag="qT")
                kT_sb = apool.tile([128, nct, t], F32R, tag="kT")
                for dst, w_sb, b_sb in ((qT_sb, wq_sb, bq_sb), (kT_sb, wk_sb, bk_sb)):
                    for m in range(nct):
                        ps = pp.tile([128, t], F32, tag="ps")
                        for k in range(nct):
                            nc.tensor.matmul(
                                ps,
                                w_sb[:, k, m * 128 : (m + 1) * 128],
                                x_sb[:, k, :],
                                start=(k == 0),
                                stop=(k == nct - 1),
                            )
                        nc.scalar.activation(
                            dst[:, m, :], ps, AF.Identity, bias=b_sb[:, m : m + 1]
                        )

                # ---- V projection (bf16) -> natural [t, c] layout + ones col ----
                v_sb = apool.tile([128, ktt, h, D + 1], BF16, tag="v")
                nc.vector.memset(v_sb[:, :, :, D : D + 1], 1.0)
                for tt in range(ktt):
                    for ch in range(nch):
                        cw = min(512, c - ch * 512)
                        ps = pp.tile([128, cw], F32, tag="ps")
                        for k in range(nct):
                            nc.tensor.matmul(
                                ps,
                                xb_sb[:, k, tt * 128 : (tt + 1) * 128],
                                wv_sb[:, k, ch * 512 : ch * 512 + cw],
                                start=(k == 0),
                                stop=(k == nct - 1),
                            )
                        nc.vector.tensor_copy(
                            v_sb[:, tt, ch * 8 : ch * 8 + cw // D, 0:D],
                            ps.rearrange("p (hh d) -> p hh d", d=D),
                        )

                # ---- attention: 4-stage pair pipeline + proj interleave ----
                yT_sb = apool.tile([128, nct, t], BF16, tag="yT", bufs=2)

                def stage_b(ps_avs, ats, ct):
                    out = []
                    for sub in range(2):
                        for i in range(ktt):
                            n = t - 128 * i
                            nc.tensor.matmul(
                                ps_avs[sub][:, 128 * i : t],
                                v_sb[:, i, 2 * ct + sub, :],
                                ats[i][:, sub, 0:n],
                                start=(i == 0),
                                stop=(i == ktt - 1),
                            )
                    for sub in range(2):
                        po = sub * 64
                        nc.vector.tensor_copy(
                            yT_sb[po : po + 64, ct, :], ps_avs[sub][0:D, :]
                        )
                        den = npool.tile([1, t], F32, tag="den", bufs=3)
                        nc.scalar.copy(den, ps_avs[sub][D : D + 1, :])
                        out.append(den)
                    return (out, ct)

                def stage_c(dens, ct):
                    for sub in range(2):
                        den = dens[sub]
                        hh = 2 * ct + sub
                        nc.sync.dma_start(scr[b, hh, 0:1, :], den[:])
                        denT = npool.tile([128, t // 128], F32, tag="denT", bufs=4)
                        nc.gpsimd.dma_start(
                            denT,
                            bass.AP(
                                tensor=scr,
                                offset=((b * h + hh) * 2) * t,
                                ap=[[1, 128], [128, t // 128]],
                            ),
                        )
                        recT = npool.tile([128, t // 128], F32, tag="recT", bufs=4)
                        nc.vector.reciprocal(recT, denT)
                        nc.sync.dma_start(
                            bass.AP(
                                tensor=scr,
                                offset=((b * h + hh) * 2 + 1) * t,
                                ap=[[1, 128], [128, t // 128]],
                            ),
                            recT,
                        )
                    rb = npool.tile([128, t], F32, tag="rb", bufs=2)
                    for sub in range(2):
                        hh = 2 * ct + sub
                        po = sub * 64
                        nc.gpsimd.dma_start(
                            rb[po : po + 64, :],
                            bass.AP(
                                tensor=scr,
                                offset=((b * h + hh) * 2 + 1) * t,
                                ap=[[0, 64], [1, t]],
                            ),
                        )
                    return (rb, ct)

                def stage_d(rb, ct):
                    for sub in range(2):
                        po = sub * 64
                        eng = nc.vector if sub == 0 else nc.gpsimd
                        eng.tensor_tensor(
                            yT_sb[po : po + 64, ct, :],
                            yT_sb[po : po + 64, ct, :],
                            rb[po : po + 64, :],
                            op=OP.mult,
                        )

                pend_b = pend_c = pend_d = None
                for ct in range(nct):  # head pair (2*ct, 2*ct+1)
                    ps_avs = [
                        pp.tile([D + 1, t], F32, tag="av", bufs=4, name=f"av{b}_{ct}_{s2}")
                        for s2 in range(2)
                    ]
                    ats = []
                    for i in range(ktt):
                        n = t - 128 * i
                        at = attp.tile([128, 2, t], BF16, tag="at")
                        for sub in range(2):
                            po = sub * 64
                            ps_s = pp.tile([128, n], F32, tag="ps")
                            nc.tensor.matmul(
                                ps_s,
                                kT_sb[po : po + 64, ct, 128 * i : 128 * (i + 1)],
                                qT_sb[po : po + 64, ct, 128 * i : t],
                                start=True,
                                stop=True,
                            )
                            nc.scalar.activation(
                                at[:, sub, 0:n],
                                ps_s,
                                AF.Exp,
                                bias=pad_sb[:, b * ktt + i : b * ktt + i + 1],
                                scale=scale,
                            )
                        eng = nc.vector if i % 2 == 0 else nc.gpsimd
                        eng.tensor_tensor(
                            at[:, :, 0:128], at[:, :, 0:128], cm_sb, op=OP.mult
                        )
                        ats.append(at)
                    # one deferred projection group of the previous batch per slot
                    if pending_proj is not None and ct >= 2 and pending_proj[2] < nct:
                        emit_proj_group(pending_proj[0], pending_proj[1], pending_proj[2])
                        pending_proj = (pending_proj[0], pending_proj[1], pending_proj[2] + 1)
                    if pend_d is not None:
                        stage_d(*pend_d)
                    pend_d = stage_c(*pend_c) if pend_c is not None else None
                    pend_c = stage_b(*pend_b) if pend_b is not None else None
                    pend_b = (ps_avs, ats, ct)
                # drain the attention pipeline
                if pend_d is not None:
                    stage_d(*pend_d)
                pend_d = stage_c(*pend_c) if pend_c is not None else None
                pend_c = stage_b(*pend_b)
                if pend_d is not None:
                    stage_d(*pend_d)
                pend_d = stage_c(*pend_c)
                stage_d(*pend_d)
                # any proj groups of the previous batch not yet emitted
                if pending_proj is not None:
                    for m in range(pending_proj[2], nct):
                        emit_proj_group(pending_proj[0], pending_proj[1], m)
                pending_proj = (yT_sb, b, 0)

            for m in range(pending_proj[2], nct):
                emit_proj_group(pending_proj[0], pending_proj[1], m)

    nc.compile()
    return nc


def _prep_core_inputs(x_local, kpm_local, c=C, t=T, bl=BL):
    """Host-side packing of one core's inputs."""
    ktt = t // 128
    xT = np.ascontiguousarray(
        x_local.transpose(2, 0, 1).reshape(c, bl * t), dtype=np.float32
    )
    pad = np.where(kpm_local, np.float32(NEG), np.float32(0.0)).astype(np.float32)
    # pad_t[p, b*ktt + i] = pad[b, i*128 + p]
    pad_t = np.ascontiguousarray(
        pad.reshape(bl, ktt, 128).transpose(2, 0, 1).reshape(128, bl * ktt)
    )
    return {"xT": xT, "xTb": xT.astype(BF16_NP), "pad_t": pad_t}


def _prep_shared_inputs(Wq, bq, Wk, bk, Wv, bv, Wp, bp, c=C):
    nct = c // 128
    Wq = np.asarray(Wq, dtype=np.float32)
    Wk = np.asarray(Wk, dtype=np.float32)
    Wv = np.asarray(Wv, dtype=np.float32)
    Wp = np.asarray(Wp, dtype=np.float32)
    bq = np.asarray(bq, dtype=np.float32)
    bk = np.asarray(bk, dtype=np.float32)
    bv = np.asarray(bv, dtype=np.float32)
    bp = np.asarray(bp, dtype=np.float32)
    bp_eff = bp + Wp @ bv
    # causal 0/1 mask for a diagonal 128x128 block in [k, q] layout
    cm1 = (np.arange(128)[:, None] <= np.arange(128)[None, :]).astype(BF16_NP)
    cm = np.ascontiguousarray(np.stack([cm1, cm1], axis=1))

    def btile(v):
        return np.ascontiguousarray(v.reshape(nct, 128).T)

    return {
        "wq_t": np.ascontiguousarray(Wq.T),
        "wk_t": np.ascontiguousarray(Wk.T),
        "wv_t": np.ascontiguousarray(Wv.T.astype(BF16_NP)),
        "wp_t": np.ascontiguousarray(Wp.T.astype(BF16_NP)),
        "bq_t": btile(bq),
        "bk_t": btile(bk),
        "bpe_t": btile(bp_eff),
        "cmask": cm,
    }


_NC_CACHE = {}


def _get_nc(key=(C, T, BL, H)):
    if key not in _NC_CACHE:
        _NC_CACHE[key] = build_nc(*key)
    return _NC_CACHE[key]


def kernel(x, key_padding_mask, Wq, bq, Wk, bk, Wv, bv, Wp, bp):
    from concourse.bass_utils import run_bass_kernel_spmd

    x = np.asarray(x, dtype=np.float32)
    kpm = np.asarray(key_padding_mask).astype(bool)

    shared = _prep_shared_inputs(Wq, bq, Wk, bk, Wv, bv, Wp, bp)
    in_maps = []
    for cid in range(N_CORES):
        sl = slice(cid * BL, (cid + 1) * BL)
        m = _prep_core_inputs(x[sl], kpm[sl])
        m.update(shared)
        in_maps.append(m)

    nc = _get_nc()
    res = run_bass_kernel_spmd(nc, in_maps, core_ids=list(range(N_CORES)))

    out = np.empty((B, T, C), dtype=np.float32)
    for cid in range(N_CORES):
        o = res.results[cid]["outT"]  # [BL, C, T]
        out[cid * BL : (cid + 1) * BL] = o.transpose(0, 2, 1)
    return out



# revision 5
# speedup vs baseline: 1.0032x; 1.0032x over previous
"""Causal multi-head self-attention (B=32, T=512, C=1024, H=16) on 8 trn2 cores.

Data-parallel over batch (4 items/core), identical NEFF on all cores.
All activations stay in [channel, token] layout so every matmul has its
contraction dim on partitions with no transposes:

  qT/kT = Wq/Wk-tiles.T @ xT-tiles          (bf16, 512-wide free dim)
  S_T   = K_slice.T @ Q_slice  [k, q]       (bf16; causal => shrink N per kt;
                                             head pairs in PE row groups)
  att   = exp(scale*S_T + pad_bias[k])      (ACT; pad mask as per-part. bias)
  att  *= causal_binmask (diag block only)  (DVE/GpSimd, 0/1 bf16 multiply)
  y/den = [V | 1].T @ att                   (bf16; ones column -> denom)
  rec   = 1/den                             (DVE reciprocal, [1,512] from PSUM)
  rb    = bcast(rec)                        (GpSimd partition_broadcast, SBUF)
  yT    = av_psum * rb                      (DVE, fused evac+normalize)
  outT  = Wp-tiles.T @ yT + bp_eff          (bf16; bv folded into bp_eff)

Differences vs the earlier version: everything bf16 (halves DMA + removes
the fp32r small-N penalty), per-k-tile weight/x tiles so the first matmul
only waits on its own DMA, no DRAM round-trip in the softmax-normalize
chain, proj-fill of the previous batch emitted FIRST in each head-pair
iteration, bf16 output DMA.
"""

import sys

sys.path.insert(0, "/opt/trn_rl_repo")

import ml_dtypes
import numpy as np

import concourse.bass as bass
import concourse.tile as tile
from concourse import bacc, mybir

B, T, C, H = 32, 512, 1024, 16
D = C // H  # 64
N_CORES = 8
BL = B // N_CORES  # batches per core
NEG = -1.0e9

F32 = mybir.dt.float32
BF16 = mybir.dt.bfloat16
BF16_NP = ml_dtypes.bfloat16
AF = mybir.ActivationFunctionType
OP = mybir.AluOpType


def build_nc(c=C, t=T, bl=BL, h=H):
    """Build the per-core Bass program. Same NEFF runs on every core."""
    nct = c // 128   # channel tiles
    ktt = t // 128   # key/token tiles per sequence
    nch = (c + 511) // 512  # 512-wide output chunks for V projection
    scale = 1.0 / float(np.sqrt(D))

    nc = bacc.Bacc(None, target_bir_lowering=False)

    xTb = nc.dram_tensor("xTb", [c, bl * t], BF16, kind="ExternalInput")
    wq_t = nc.dram_tensor("wq_t", [c, c], BF16, kind="ExternalInput")
    wk_t = nc.dram_tensor("wk_t", [c, c], BF16, kind="ExternalInput")
    wv_t = nc.dram_tensor("wv_t", [c, c], BF16, kind="ExternalInput")
    wp_t = nc.dram_tensor("wp_t", [c, c], BF16, kind="ExternalInput")
    bq_t = nc.dram_tensor("bq_t", [128, nct], F32, kind="ExternalInput")
    bk_t = nc.dram_tensor("bk_t", [128, nct], F32, kind="ExternalInput")
    bpe_t = nc.dram_tensor("bpe_t", [128, nct], F32, kind="ExternalInput")
    pad_t = nc.dram_tensor("pad_t", [128, bl * ktt], F32, kind="ExternalInput")
    cmask = nc.dram_tensor("cmask", [128, 2, 128], BF16, kind="ExternalInput")
    outT = nc.dram_tensor("outT", [bl, c, t], BF16, kind="ExternalOutput")

    with tile.TileContext(nc) as tc:
        with (
            tc.tile_pool(name="weights", bufs=1) as wpool,
            tc.tile_pool(name="consts", bufs=1) as cpool,
            tc.tile_pool(name="acts", bufs=1) as apool,
            tc.tile_pool(name="att", bufs=8) as attp,
            tc.tile_pool(name="norm", bufs=2) as npool,
            tc.tile_pool(name="oevac", bufs=2) as opool,
            tc.tile_pool(name="psum", bufs=2, space=bass.MemorySpace.PSUM) as pp,
        ):
            # ---- constants first (tiny), then wq, then x(b0), then wk/wv/wp
            bq_sb = cpool.tile([128, nct], F32, tag="bq")
            bk_sb = cpool.tile([128, nct], F32, tag="bk")
            bpe_sb = cpool.tile([128, nct], F32, tag="bpe")
            pad_sb = cpool.tile([128, bl * ktt], F32, tag="pad")
            cm_sb = cpool.tile([128, 2, 128], BF16, tag="cmask")
            nc.sync.dma_start(bq_sb, bq_t[:])
            nc.sync.dma_start(bk_sb, bk_t[:])
            nc.sync.dma_start(bpe_sb, bpe_t[:])
            nc.sync.dma_start(pad_sb, pad_t[:])
            nc.sync.dma_start(cm_sb, cmask[:])

            def load_w(dram, tag):
                tiles = [
                    wpool.tile([128, c], BF16, tag=f"{tag}{k}", name=f"{tag}{k}")
                    for k in range(nct)
                ]
                w_r = dram[:].rearrange("(k p) m -> p k m", p=128)
                for k in range(nct):
                    nc.sync.dma_start(tiles[k], w_r[:, k, :])
                return tiles

            wq_k = load_w(wq_t, "wq")

            # x(b0) before the remaining weights so Q-proj starts early
            def load_x(b):
                tiles = [
                    apool.tile([128, t], BF16, tag=f"x{k}", bufs=2, name=f"x{b}_{k}")
                    for k in range(nct)
                ]
                x_r = xTb[:, b * t : (b + 1) * t].rearrange(
                    "(k p) n -> p k n", p=128
                )
                for k in range(nct):
                    nc.sync.dma_start(tiles[k], x_r[:, k, :])
                return tiles

            x_k = load_x(0)
            wk_k = load_w(wk_t, "wk")
            wv_k = load_w(wv_t, "wv")
            wp_k = load_w(wp_t, "wp")

            # per-m qT/kT tiles (fine-grained deps for the S matmuls)
            qT = [
                apool.tile([128, t], BF16, tag=f"qT{m}", name=f"qT{m}")
                for m in range(nct)
            ]
            kT = [
                apool.tile([128, t], BF16, tag=f"kT{m}", name=f"kT{m}")
                for m in range(nct)
            ]

            def emit_proj_group(yT_sb, b, m):
                ps = pp.tile([128, t], F32, tag="ps", name=f"pj{b}_{m}")
                for k in range(nct):
                    nc.tensor.matmul(
                        ps,
                        wp_k[k][:, m * 128 : (m + 1) * 128],
                        yT_sb[:, k, :],
                        start=(k == 0),
                        stop=(k == nct - 1),
                    )
                ot = opool.tile([128, t], BF16, tag="ot", name=f"ot{b}_{m}")
                nc.scalar.activation(
                    ot, ps, AF.Identity, bias=bpe_sb[:, m : m + 1]
                )
                nc.sync.dma_start(outT[b, m * 128 : (m + 1) * 128, :], ot)

            pending_proj = None  # (yT_sb, b, next_m) — interleaved into attn

            for b in range(bl):
                if b > 0:
                    x_k = load_x(b)

                # ---- Q/K projections (bf16) -> per-m [128, t] tiles ----
                for dst, w_tiles, b_sb in (
                    (qT, wq_k, bq_sb),
                    (kT, wk_k, bk_sb),
                ):
                    for m in range(nct):
                        ps = pp.tile([128, t], F32, tag="ps")
                        for k in range(nct):
                            nc.tensor.matmul(
                                ps,
                                w_tiles[k][:, m * 128 : (m + 1) * 128],
                                x_k[k],
                                start=(k == 0),
                                stop=(k == nct - 1),
                            )
                        nc.scalar.activation(
                            dst[m], ps, AF.Identity, bias=b_sb[:, m : m + 1]
                        )

                # ---- V projection (bf16) -> [t, c] layout + ones column ----
                v_sb = apool.tile([128, ktt, h, D + 1], BF16, tag="v", bufs=2)
                nc.vector.memset(v_sb[:, :, :, D : D + 1], 1.0)
                for tt in range(ktt):
                    for ch in range(nch):
                        cw = min(512, c - ch * 512)
                        ps = pp.tile([128, cw], F32, tag="ps")
                        for k in range(nct):
                            nc.tensor.matmul(
                                ps,
                                x_k[k][:, tt * 128 : (tt + 1) * 128],
                                wv_k[k][:, ch * 512 : ch * 512 + cw],
                                start=(k == 0),
                                stop=(k == nct - 1),
                            )
                        nc.vector.tensor_copy(
                            v_sb[:, tt, ch * 8 : ch * 8 + cw // D, 0:D],
                            ps.rearrange("p (hh d) -> p hh d", d=D),
                        )

                # ---- attention: 1-deep pair pipeline + proj interleave ----
                yT_sb = apool.tile([128, nct, t], BF16, tag="yT", bufs=2)

                def stage_bc(ats, ct):
                    # AV matmuls for head pair ct (ones column -> denom row D)
                    avs = [
                        pp.tile([D + 1, t], F32, tag="av", name=f"av{b}_{ct}_{s2}")
                        for s2 in range(2)
                    ]
                    for sub in range(2):
                        for i in range(ktt):
                            n = t - 128 * i
                            nc.tensor.matmul(
                                avs[sub][:, 128 * i : t],
                                v_sb[:, i, 2 * ct + sub, :],
                                ats[i][:, sub, 0:n],
                                start=(i == 0),
                                stop=(i == ktt - 1),
                            )
                    # reciprocal of the denominators, on-chip broadcast,
                    # fused evacuate+normalize into yT
                    rec = npool.tile([1, 2, t], F32, tag="rec", bufs=2)
                    for sub in range(2):
                        nc.vector.reciprocal(
                            rec[:, sub, :], avs[sub][D : D + 1, :]
                        )
                    # one broadcast to all 128 partitions (gpsimd ucode
                    # requires the output to start at partition 0)
                    rb = npool.tile([128, 2, t], F32, tag="rb", bufs=2)
                    nc.gpsimd.partition_broadcast(rb, rec, channels=128)
                    for sub in range(2):
                        po = sub * 64
                        nc.vector.tensor_tensor(
                            yT_sb[po : po + 64, ct, :],
                            avs[sub][0:D, :],
                            rb[po : po + 64, sub, :],
                            op=OP.mult,
                        )

                pend = None
                for ct in range(nct):  # head pair (2*ct, 2*ct+1)
                    # proj-fill of previous batch FIRST: deps always ready
                    if (
                        pending_proj is not None
                        and ct >= 1
                        and pending_proj[2] < nct
                    ):
                        emit_proj_group(
                            pending_proj[0], pending_proj[1], pending_proj[2]
                        )
                        pending_proj = (
                            pending_proj[0],
                            pending_proj[1],
                            pending_proj[2] + 1,
                        )
                    ats = []
                    for i in range(ktt):
                        n = t - 128 * i
                        at = attp.tile([128, 2, t], BF16, tag="at")
                        for sub in range(2):
                            po = sub * 64
                            ps_s = pp.tile([128, n], F32, tag="ss", bufs=4)
                            nc.tensor.matmul(
                                ps_s,
                                kT[ct][po : po + 64, 128 * i : 128 * (i + 1)],
                                qT[ct][po : po + 64, 128 * i : t],
                                start=True,
                                stop=True,
                            )
                            nc.scalar.activation(
                                at[:, sub, 0:n],
                                ps_s,
                                AF.Exp,
                                bias=pad_sb[:, b * ktt + i : b * ktt + i + 1],
                                scale=scale,
                            )
                        eng = nc.vector if i % 2 == 0 else nc.gpsimd
                        eng.tensor_tensor(
                            at[:, :, 0:128], at[:, :, 0:128], cm_sb, op=OP.mult
                        )
                        ats.append(at)
                    if pend is not None:
                        stage_bc(*pend)
                    pend = (ats, ct)
                stage_bc(*pend)  # drain
                # any proj groups of the previous batch not yet emitted
                if pending_proj is not None:
                    for m in range(pending_proj[2], nct):
                        emit_proj_group(pending_proj[0], pending_proj[1], m)
                pending_proj = (yT_sb, b, 0)

            for m in range(pending_proj[2], nct):
                emit_proj_group(pending_proj[0], pending_proj[1], m)

    nc.compile()
    return nc


def _prep_core_inputs(x_local, kpm_local, c=C, t=T, bl=BL):
    """Host-side packing of one core's inputs."""
    ktt = t // 128
    xT = np.ascontiguousarray(
        np.asarray(x_local, dtype=np.float32).transpose(2, 0, 1).reshape(c, bl * t)
    )
    pad = np.where(kpm_local, np.float32(NEG), np.float32(0.0)).astype(np.float32)
    # pad_t[p, b*ktt + i] = pad[b, i*128 + p]
    pad_t = np.ascontiguousarray(
        pad.reshape(bl, ktt, 128).transpose(2, 0, 1).reshape(128, bl * ktt)
    )
    return {"xTb": xT.astype(BF16_NP), "pad_t": pad_t}


def _prep_shared_inputs(Wq, bq, Wk, bk, Wv, bv, Wp, bp, c=C):
    nct = c // 128
    Wq = np.asarray(Wq, dtype=np.float32)
    Wk = np.asarray(Wk, dtype=np.float32)
    Wv = np.asarray(Wv, dtype=np.float32)
    Wp = np.asarray(Wp, dtype=np.float32)
    bq = np.asarray(bq, dtype=np.float32)
    bk = np.asarray(bk, dtype=np.float32)
    bv = np.asarray(bv, dtype=np.float32)
    bp = np.asarray(bp, dtype=np.float32)
    bp_eff = bp + Wp @ bv
    # causal 0/1 mask for a diagonal 128x128 block in [k, q] layout
    cm1 = (np.arange(128)[:, None] <= np.arange(128)[None, :]).astype(BF16_NP)
    cm = np.ascontiguousarray(np.stack([cm1, cm1], axis=1))

    def btile(v):
        return np.ascontiguousarray(v.reshape(nct, 128).T)

    return {
        "wq_t": np.ascontiguousarray(Wq.T.astype(BF16_NP)),
        "wk_t": np.ascontiguousarray(Wk.T.astype(BF16_NP)),
        "wv_t": np.ascontiguousarray(Wv.T.astype(BF16_NP)),
        "wp_t": np.ascontiguousarray(Wp.T.astype(BF16_NP)),
        "bq_t": btile(bq),
        "bk_t": btile(bk),
        "bpe_t": btile(bp_eff),
        "cmask": cm,
    }


_NC_CACHE = {}


def _get_nc(key=(C, T, BL, H)):
    if key not in _NC_CACHE:
        _NC_CACHE[key] = build_nc(*key)
    return _NC_CACHE[key]


LAST_RESULT = None  # test harness reads exec_time_ns / trace path from here


def kernel(
    x, key_padding_mask, Wq, bq, Wk, bk, Wv, bv, Wp, bp,
    _trace=False, _trace_kwargs=None,
):
    global LAST_RESULT
    from concourse.bass_utils import run_bass_kernel_spmd

    x = np.asarray(x, dtype=np.float32)
    kpm = np.asarray(key_padding_mask).astype(bool)

    shared = _prep_shared_inputs(Wq, bq, Wk, bk, Wv, bv, Wp, bp)
    in_maps = []
    for cid in range(N_CORES):
        sl = slice(cid * BL, (cid + 1) * BL)
        m = _prep_core_inputs(x[sl], kpm[sl])
        m.update(shared)
        in_maps.append(m)

    nc = _get_nc()
    kw = {}
    if _trace:
        kw = dict(trace=True, trace_cores=[0], trace_kwargs=_trace_kwargs or {})
    res = run_bass_kernel_spmd(nc, in_maps, core_ids=list(range(N_CORES)), **kw)
    LAST_RESULT = res

    out = np.empty((B, T, C), dtype=np.float32)
    for cid in range(N_CORES):
        o = np.asarray(res.results[cid]["outT"], dtype=np.float32)  # [BL, C, T]
        out[cid * BL : (cid + 1) * BL] = o.transpose(0, 2, 1)
    return out


# revision 7
# speedup vs baseline: 1.3153x; 1.3112x over previous
"""Causal multi-head self-attention (B=32, T=512, C=1024, H=16) on 8 trn2 cores.

Data-parallel over batch (4 items/core), identical NEFF on all cores.
All activations stay in [channel, token] layout so every matmul has its
contraction dim on partitions with no transposes:

  qT/kT = Wq/Wk-tiles.T @ xT-tiles          (bf16, 512-wide free dim)
  S_T   = K_slice.T @ Q_slice  [k, q]       (bf16; causal => shrink N per kt;
                                             head pairs in PE row groups)
  att   = exp(scale*S_T + pad_bias[k])      (ACT; pad mask as per-part. bias)
  att  *= causal_binmask (diag block only)  (DVE/GpSimd, 0/1 bf16 multiply)
  y/den = [V | 1].T @ att                   (bf16; ones column -> denom)
  rec   = 1/den                             (DVE reciprocal, [1,512] from PSUM)
  rb    = bcast(rec)                        (GpSimd partition_broadcast, SBUF)
  yT    = av_psum * rb                      (DVE, fused evac+normalize)
  outT  = Wp-tiles.T @ yT + bp_eff          (bf16; bv folded into bp_eff)

Differences vs the earlier version: everything bf16 (halves DMA + removes
the fp32r small-N penalty), per-k-tile weight/x tiles so the first matmul
only waits on its own DMA, no DRAM round-trip in the softmax-normalize
chain, proj-fill of the previous batch emitted FIRST in each head-pair
iteration, bf16 output DMA.
"""

import sys

sys.path.insert(0, "/opt/trn_rl_repo")

import ml_dtypes
import numpy as np

import concourse.bass as bass
import concourse.tile as tile
from concourse import bacc, mybir

B, T, C, H = 32, 512, 1024, 16
D = C // H  # 64
N_CORES = 8
BL = B // N_CORES  # batches per core
NEG = -1.0e9

F32 = mybir.dt.float32
BF16 = mybir.dt.bfloat16
BF16_NP = ml_dtypes.bfloat16
AF = mybir.ActivationFunctionType
OP = mybir.AluOpType


def build_nc(c=C, t=T, bl=BL, h=H):
    """Build the per-core Bass program. Same NEFF runs on every core."""
    nct = c // 128   # channel tiles
    ktt = t // 128   # key/token tiles per sequence
    nch = (c + 511) // 512  # 512-wide output chunks for V projection
    scale = 1.0 / float(np.sqrt(D))

    nc = bacc.Bacc(None, target_bir_lowering=False)

    xTb = nc.dram_tensor("xTb", [c, bl * t], BF16, kind="ExternalInput")
    wq_t = nc.dram_tensor("wq_t", [c, c], BF16, kind="ExternalInput")
    wk_t = nc.dram_tensor("wk_t", [c, c], BF16, kind="ExternalInput")
    wv_t = nc.dram_tensor("wv_t", [c, c], BF16, kind="ExternalInput")
    wp_t = nc.dram_tensor("wp_t", [c, c], BF16, kind="ExternalInput")
    bq_t = nc.dram_tensor("bq_t", [128, nct], F32, kind="ExternalInput")
    bk_t = nc.dram_tensor("bk_t", [128, nct], F32, kind="ExternalInput")
    bpe_t = nc.dram_tensor("bpe_t", [128, nct], F32, kind="ExternalInput")
    pad_t = nc.dram_tensor("pad_t", [128, bl * ktt], F32, kind="ExternalInput")
    cmask = nc.dram_tensor("cmask", [128, 2, 128], BF16, kind="ExternalInput")
    outT = nc.dram_tensor("outT", [bl, c, t], BF16, kind="ExternalOutput")

    with tile.TileContext(nc) as tc:
        with (
            tc.tile_pool(name="weights", bufs=1) as wpool,
            tc.tile_pool(name="consts", bufs=1) as cpool,
            tc.tile_pool(name="acts", bufs=1) as apool,
            tc.tile_pool(name="att", bufs=8) as attp,
            tc.tile_pool(name="norm", bufs=2) as npool,
            tc.tile_pool(name="oevac", bufs=2) as opool,
            tc.tile_pool(name="psum", bufs=2, space=bass.MemorySpace.PSUM) as pp,
        ):
            # ---- constants first (tiny), then wq, then x(b0), then wk/wv/wp
            bq_sb = cpool.tile([128, nct], F32, tag="bq")
            bk_sb = cpool.tile([128, nct], F32, tag="bk")
            bpe_sb = cpool.tile([128, nct], F32, tag="bpe")
            pad_sb = cpool.tile([128, bl * ktt], F32, tag="pad")
            cm_sb = cpool.tile([128, 2, 128], BF16, tag="cmask")
            nc.sync.dma_start(bq_sb, bq_t[:])
            nc.sync.dma_start(bk_sb, bk_t[:])
            nc.sync.dma_start(bpe_sb, bpe_t[:])
            nc.sync.dma_start(pad_sb, pad_t[:])
            nc.sync.dma_start(cm_sb, cmask[:])

            def load_w(dram, tag):
                tiles = [
                    wpool.tile([128, c], BF16, tag=f"{tag}{k}", name=f"{tag}{k}")
                    for k in range(nct)
                ]
                w_r = dram[:].rearrange("(k p) m -> p k m", p=128)
                for k in range(nct):
                    nc.sync.dma_start(tiles[k], w_r[:, k, :])
                return tiles

            wq_k = load_w(wq_t, "wq")

            # x(b0) before the remaining weights so Q-proj starts early
            def load_x(b):
                tiles = [
                    apool.tile([128, t], BF16, tag=f"x{k}", bufs=2, name=f"x{b}_{k}")
                    for k in range(nct)
                ]
                x_r = xTb[:, b * t : (b + 1) * t].rearrange(
                    "(k p) n -> p k n", p=128
                )
                for k in range(nct):
                    nc.sync.dma_start(tiles[k], x_r[:, k, :])
                return tiles

            x_k = load_x(0)
            wk_k = load_w(wk_t, "wk")
            wv_k = load_w(wv_t, "wv")
            wp_k = load_w(wp_t, "wp")

            # per-m qT/kT tiles (fine-grained deps for the S matmuls)
            qT = [
                apool.tile([128, t], BF16, tag=f"qT{m}", name=f"qT{m}")
                for m in range(nct)
            ]
            kT = [
                apool.tile([128, t], BF16, tag=f"kT{m}", name=f"kT{m}")
                for m in range(nct)
            ]

            def emit_proj_group(yT_sb, b, m):
                ps = pp.tile([128, t], F32, tag="ps", name=f"pj{b}_{m}")
                for k in range(nct):
                    nc.tensor.matmul(
                        ps,
                        wp_k[k][:, m * 128 : (m + 1) * 128],
                        yT_sb[:, k, :],
                        start=(k == 0),
                        stop=(k == nct - 1),
                    )
                ot = opool.tile([128, t], BF16, tag="ot", name=f"ot{b}_{m}")
                nc.scalar.activation(
                    ot, ps, AF.Identity, bias=bpe_sb[:, m : m + 1]
                )
                nc.sync.dma_start(outT[b, m * 128 : (m + 1) * 128, :], ot)

            pending_proj = None  # (yT_sb, b, next_m) — interleaved into attn

            for b in range(bl):
                if b > 0:
                    x_k = load_x(b)

                # ---- Q/K projections (bf16) -> per-m [128, t] tiles ----
                for dst, w_tiles, b_sb in (
                    (qT, wq_k, bq_sb),
                    (kT, wk_k, bk_sb),
                ):
                    for m in range(nct):
                        ps = pp.tile([128, t], F32, tag="ps")
                        for k in range(nct):
                            nc.tensor.matmul(
                                ps,
                                w_tiles[k][:, m * 128 : (m + 1) * 128],
                                x_k[k],
                                start=(k == 0),
                                stop=(k == nct - 1),
                            )
                        nc.scalar.activation(
                            dst[m], ps, AF.Identity, bias=b_sb[:, m : m + 1]
                        )

                # ---- V projection (bf16) -> [t, c] layout + ones column ----
                v_sb = apool.tile([128, ktt, h, D + 1], BF16, tag="v", bufs=2)
                nc.vector.memset(v_sb[:, :, :, D : D + 1], 1.0)
                for tt in range(ktt):
                    for ch in range(nch):
                        cw = min(512, c - ch * 512)
                        ps = pp.tile([128, cw], F32, tag="ps")
                        for k in range(nct):
                            nc.tensor.matmul(
                                ps,
                                x_k[k][:, tt * 128 : (tt + 1) * 128],
                                wv_k[k][:, ch * 512 : ch * 512 + cw],
                                start=(k == 0),
                                stop=(k == nct - 1),
                            )
                        nc.vector.tensor_copy(
                            v_sb[:, tt, ch * 8 : ch * 8 + cw // D, 0:D],
                            ps.rearrange("p (hh d) -> p hh d", d=D),
                        )

                # ---- attention: 1-deep pair pipeline + proj interleave ----
                yT_sb = apool.tile([128, nct, t], BF16, tag="yT", bufs=2)

                def stage_bc(ats, ct):
                    # AV matmuls for head pair ct (ones column -> denom row D)
                    avs = [
                        pp.tile([D + 1, t], F32, tag="av", name=f"av{b}_{ct}_{s2}")
                        for s2 in range(2)
                    ]
                    for sub in range(2):
                        for i in range(ktt):
                            n = t - 128 * i
                            nc.tensor.matmul(
                                avs[sub][:, 128 * i : t],
                                v_sb[:, i, 2 * ct + sub, :],
                                ats[i][:, sub, 0:n],
                                start=(i == 0),
                                stop=(i == ktt - 1),
                            )
                    # reciprocal of the denominators, on-chip broadcast,
                    # fused evacuate+normalize into yT
                    # approx-recip needs an SBUF source; evacuate the two
                    # denominator rows on ACT/DVE then one fused recip op
                    den = npool.tile([1, 2, t], F32, tag="den", bufs=2)
                    nc.scalar.copy(den[:, 0, :], avs[0][D : D + 1, :])
                    nc.vector.tensor_copy(den[:, 1, :], avs[1][D : D + 1, :])
                    rec = npool.tile([1, 2, t], F32, tag="rec", bufs=2)
                    nc.vector.reciprocal_approx_fast(out=rec, in_=den)
                    # one broadcast to all 128 partitions (gpsimd ucode
                    # requires the output to start at partition 0)
                    rb = npool.tile([128, 2, t], F32, tag="rb", bufs=2)
                    nc.gpsimd.partition_broadcast(rb, rec, channels=128)
                    for sub in range(2):
                        po = sub * 64
                        nc.vector.tensor_tensor(
                            yT_sb[po : po + 64, ct, :],
                            avs[sub][0:D, :],
                            rb[po : po + 64, sub, :],
                            op=OP.mult,
                        )

                pend = None
                for ct in range(nct):  # head pair (2*ct, 2*ct+1)
                    # proj-fill of previous batch FIRST: deps always ready
                    if (
                        pending_proj is not None
                        and ct >= 1
                        and pending_proj[2] < nct
                    ):
                        emit_proj_group(
                            pending_proj[0], pending_proj[1], pending_proj[2]
                        )
                        pending_proj = (
                            pending_proj[0],
                            pending_proj[1],
                            pending_proj[2] + 1,
                        )
                    ats = []
                    for i in range(ktt):
                        n = t - 128 * i
                        at = attp.tile([128, 2, t], BF16, tag="at")
                        for sub in range(2):
                            po = sub * 64
                            ps_s = pp.tile([128, n], F32, tag="ss", bufs=4)
                            nc.tensor.matmul(
                                ps_s,
                                kT[ct][po : po + 64, 128 * i : 128 * (i + 1)],
                                qT[ct][po : po + 64, 128 * i : t],
                                start=True,
                                stop=True,
                            )
                            nc.scalar.activation(
                                at[:, sub, 0:n],
                                ps_s,
                                AF.Exp,
                                bias=pad_sb[:, b * ktt + i : b * ktt + i + 1],
                                scale=scale,
                            )
                        eng = nc.vector if i % 2 == 0 else nc.gpsimd
                        eng.tensor_tensor(
                            at[:, :, 0:128], at[:, :, 0:128], cm_sb, op=OP.mult
                        )
                        ats.append(at)
                    if pend is not None:
                        stage_bc(*pend)
                    pend = (ats, ct)
                stage_bc(*pend)  # drain
                # any proj groups of the previous batch not yet emitted
                if pending_proj is not None:
                    for m in range(pending_proj[2], nct):
                        emit_proj_group(pending_proj[0], pending_proj[1], m)
                pending_proj = (yT_sb, b, 0)

            for m in range(pending_proj[2], nct):
                emit_proj_group(pending_proj[0], pending_proj[1], m)

    nc.compile()
    return nc


def _prep_core_inputs(x_local, kpm_local, c=C, t=T, bl=BL):
    """Host-side packing of one core's inputs."""
    ktt = t // 128
    xT = np.ascontiguousarray(
        np.asarray(x_local, dtype=np.float32).transpose(2, 0, 1).reshape(c, bl * t)
    )
    pad = np.where(kpm_local, np.float32(NEG), np.float32(0.0)).astype(np.float32)
    # pad_t[p, b*ktt + i] = pad[b, i*128 + p]
    pad_t = np.ascontiguousarray(
        pad.reshape(bl, ktt, 128).transpose(2, 0, 1).reshape(128, bl * ktt)
    )
    return {"xTb": xT.astype(BF16_NP), "pad_t": pad_t}


def _prep_shared_inputs(Wq, bq, Wk, bk, Wv, bv, Wp, bp, c=C):
    nct = c // 128
    Wq = np.asarray(Wq, dtype=np.float32)
    Wk = np.asarray(Wk, dtype=np.float32)
    Wv = np.asarray(Wv, dtype=np.float32)
    Wp = np.asarray(Wp, dtype=np.float32)
    bq = np.asarray(bq, dtype=np.float32)
    bk = np.asarray(bk, dtype=np.float32)
    bv = np.asarray(bv, dtype=np.float32)
    bp = np.asarray(bp, dtype=np.float32)
    bp_eff = bp + Wp @ bv
    # causal 0/1 mask for a diagonal 128x128 block in [k, q] layout
    cm1 = (np.arange(128)[:, None] <= np.arange(128)[None, :]).astype(BF16_NP)
    cm = np.ascontiguousarray(np.stack([cm1, cm1], axis=1))

    def btile(v):
        return np.ascontiguousarray(v.reshape(nct, 128).T)

    return {
        "wq_t": np.ascontiguousarray(Wq.T.astype(BF16_NP)),
        "wk_t": np.ascontiguousarray(Wk.T.astype(BF16_NP)),
        "wv_t": np.ascontiguousarray(Wv.T.astype(BF16_NP)),
        "wp_t": np.ascontiguousarray(Wp.T.astype(BF16_NP)),
        "bq_t": btile(bq),
        "bk_t": btile(bk),
        "bpe_t": btile(bp_eff),
        "cmask": cm,
    }


_NC_CACHE = {}


def _get_nc(key=(C, T, BL, H)):
    if key not in _NC_CACHE:
        _NC_CACHE[key] = build_nc(*key)
    return _NC_CACHE[key]


LAST_RESULT = None  # test harness reads exec_time_ns / trace path from here


def kernel(
    x, key_padding_mask, Wq, bq, Wk, bk, Wv, bv, Wp, bp,
    _trace=False, _trace_kwargs=None,
):
    global LAST_RESULT
    from concourse.bass_utils import run_bass_kernel_spmd

    x = np.asarray(x, dtype=np.float32)
    kpm = np.asarray(key_padding_mask).astype(bool)

    shared = _prep_shared_inputs(Wq, bq, Wk, bk, Wv, bv, Wp, bp)
    in_maps = []
    for cid in range(N_CORES):
        sl = slice(cid * BL, (cid + 1) * BL)
        m = _prep_core_inputs(x[sl], kpm[sl])
        m.update(shared)
        in_maps.append(m)

    nc = _get_nc()
    kw = {}
    if _trace:
        kw = dict(trace=True, trace_cores=[0], trace_kwargs=_trace_kwargs or {})
    res = run_bass_kernel_spmd(nc, in_maps, core_ids=list(range(N_CORES)), **kw)
    LAST_RESULT = res

    out = np.empty((B, T, C), dtype=np.float32)
    for cid in range(N_CORES):
        o = np.asarray(res.results[cid]["outT"], dtype=np.float32)  # [BL, C, T]
        out[cid * BL : (cid + 1) * BL] = o.transpose(0, 2, 1)
    return out
